# revision 35
# baseline (speedup 1.0000x reference)
# Causal self-attention kernel for 8 Trainium2 NeuronCores (Bass/Tile).
#
# Sharding: core c -> batch b = c//4, head group g = c%4 (heads 4g..4g+3).
# Each core computes the qkv projection for its batch restricted to its heads
# (column-sharded Wqkv), rope, causal flash attention for its 4 heads, and a
# row-sharded output projection producing a partial [S, D] bf16 output.  The
# host sums the 4 partials per batch and adds bout.
#
# The kernel is organized as a single software-pipelined stream: the scalar
# engine's softmax exp is the attention-phase bottleneck (~80us of ACT time vs
# ~58us of attention matmul), so the qkv projection waves, v projection and
# output projection are emitted as PE "filler" units interleaved between
# attention steps instead of as separate serial phases.  Attention begins on
# the first head as soon as its q/k columns are roped (~11us in), while the
# rest of x is still loading.
#
# Device-side notes:
#  * All matmul inputs are bf16; accumulation f32 in PSUM.
#  * x is pre-transposed on host to xT [D, S]; loaded criticality-first as
#    merged 3D DMAs (seq-half 0 on sync, seq-half 1 on gpsimd); w loads in
#    column-group order [qk01 | v | qk23] on scalar so the first projection
#    waves start as early as possible.  Attention blocks alternate small
#    (half-0) and big (half-1) so projection fillers land where ACT is the
#    local bottleneck.
#  * q/k are produced directly transposed (qT/kT [dims, S]).  Within each head
#    the dims are permuted to [e0..15, o0..15, e16..31, o16..31] so each rope
#    partner lives in the same 32-partition quadrant: the even/odd swap is a
#    single DVE stream_shuffle (mask = swap 16-halves) instead of SBUF DMAs.
#  * Scores are computed transposed, sT[k, q], with the k-side stationary
#    operand zero-padded to K=128 (K=64 matmuls never warm the PE clock gate).
#  * Causal masking of diagonal score tiles is one extra accumulating matmul
#    (-30000 * upper-triangle).
#  * Softmax without max-subtraction: p = exp(s/8) straight out of PSUM on the
#    scalar engine, bf16 out.  The scalar engine does (almost) nothing else.
#  * v_ext [k, 65] carries a ones-column so the PV matmul accumulates the
#    softmax denominator as row 64 of oT [65, q].
#  * Softmax denominators are reciprocated via a DMA transpose [1,1024] ->
#    [128,8] so the (slow, serial-in-free-dim) DVE reciprocal runs across
#    partitions: ~0.1us instead of ~6.6us per head-half.
#  * y partials are written bf16 (host accumulates in f32); stores spread
#    through the attention stream instead of draining at the end.

import numpy as np
import ml_dtypes

import concourse.bass as bass
import concourse.tile as tile
import concourse.mybir as mybir
from concourse import bacc
from concourse.bass import ts, ds
from concourse.bass_utils import run_bass_kernel_spmd

BF16 = mybir.dt.bfloat16
F32 = mybir.dt.float32
AF = mybir.ActivationFunctionType
ALU = mybir.AluOpType

B, S, D = 2, 2048, 1024
H, HD = 16, 64
NCORES = 8
HPC = 4            # heads per core
NT = S // 128      # 16 seq tiles
SCALE = HD ** -0.5
MASK_NEG = -30000.0

SWAP16 = [16 + i for i in range(16)] + list(range(16))  # stream_shuffle mask

# Module-level knobs / results (used by test.py).
TRACE = False
LAST_RESULTS = None


def _body(ctx, tc, ins, outs):
    nc = tc.nc
    xh, wg, wout, bqk, bvb, ropeP, ropeQ = ins
    (y,) = outs

    # ---- SBUF pools ----
    p_x = ctx.enter_context(tc.tile_pool(name="x", bufs=1))
    p_w = ctx.enter_context(tc.tile_pool(name="w", bufs=1))
    p_cst = ctx.enter_context(tc.tile_pool(name="cst", bufs=1))
    p_qk = ctx.enter_context(tc.tile_pool(name="qk", bufs=1))
    p_vx = ctx.enter_context(tc.tile_pool(name="vx", bufs=1))
    p_aT = ctx.enter_context(tc.tile_pool(name="aT", bufs=1))
    p_tmp = ctx.enter_context(tc.tile_pool(name="tmp", bufs=3))
    p_p = ctx.enter_context(tc.tile_pool(name="p", bufs=6))
    p_r = ctx.enter_context(tc.tile_pool(name="r", bufs=2))
    p_y = ctx.enter_context(tc.tile_pool(name="y", bufs=4))

    # ---- PE warmup fodder: memset junk immediately (no DMA dependency) ----
    junk = p_cst.tile([128, 128], BF16, tag="junk")
    nc.vector.memset(junk[:, :], 0.25)

    # ---- input DMA issues ----
    # The host pre-arranges x and w into the exact SBUF layout, so every
    # load is a pure 2D contiguous-row DMA (128 descriptors, ~0.8us issue;
    # multi-row 3D APs cost ~5-9us of HWDGE issue time on the engine).
    # Rings stream descriptors in issue order and fair-share HBM, so the
    # ~4MB critical set (x half-0, w qk01 cols, rope, bqk) leads every ring.
    #   xh layout: [p, half, kc, 1024]  ->  x_t2 cols (half*8 + kc)*1024
    #   wg layout: [p, grp, kc, 256], grp 0 = q01|k01, 1 = v, 2 = q23|k23
    x_t2 = p_x.tile([128, 8 * S], BF16, tag="x", name="x")
    x4 = x_t2.rearrange("p (h kc s) -> p h kc s", kc=8, s=1024)
    w_t2 = p_w.tile([128, 3 * 2048], BF16, tag="w", name="w")
    w4 = w_t2.rearrange("p (g kc c) -> p g kc c", kc=8, c=256)
    tabP = p_cst.tile([128, S], BF16, tag="tabP")
    tabQ = p_cst.tile([128, S], BF16, tag="tabQ")
    bqk_sb = p_cst.tile([128, 4], F32, tag="bqk")
    bvb_sb = p_cst.tile([128, 256], F32, tag="bvb")
    wout_sb = p_w.tile([128, 2048], BF16, tag="wout")
    # Ring order IS the priority: within a ring, transfers complete in
    # issue order, and concurrent rings fair-share HBM.  Bulk that isn't
    # needed until late (x half-1, wout, w23) goes BEHIND the critical set
    # on the same rings instead of competing from a third ring.  gpsimd's
    # SWDGE ring stays clear for the latency-sensitive fq round-trips and
    # the y stores.
    nc.sync.dma_start(x_t2[:, 0:4096], xh[:, 0:4096])
    nc.scalar.dma_start(bqk_sb[:, :], bqk[:, :])
    nc.scalar.dma_start(w_t2[:, 0:2048], wg[:, 0:2048])
    nc.sync.dma_start(x_t2[:, 4096:8192], xh[:, 4096:8192])
    nc.scalar.dma_start(tabQ[:, 0:1024], ropeQ[:, 0:1024])
    nc.scalar.dma_start(tabP[:, 0:1024], ropeP[:, 0:1024])
    nc.scalar.dma_start(w_t2[:, 2048:4096], wg[:, 2048:4096])
    nc.scalar.dma_start(bvb_sb[:, :], bvb[:, :])
    nc.scalar.dma_start(tabQ[:, 1024:2048], ropeQ[:, 1024:2048])
    nc.scalar.dma_start(tabP[:, 1024:2048], ropeP[:, 1024:2048])
    # bulk remainder, strictly behind the critical set on the same rings
    nc.sync.dma_start(wout_sb[:, :], wout[:, :])
    nc.scalar.dma_start(x_t2[:, 8192:16384], xh[:, 8192:16384])
    nc.scalar.dma_start(w_t2[:, 4096:6144], wg[:, 4096:6144])
    # (group, col offset) in w4 for each qk projection unit column block
    GOFF = {0: (0, 0), 2: (0, 128), 1: (2, 0), 3: (2, 128)}

    # per-head zero-padded kT [128, S]: only head h's 64 rows (at offset
    # 64*(h%2)) are nonzero, for full-K scores matmuls.  The memsets run on
    # the DVE, which is otherwise idle until the first projection evacuates
    # (~14us); gpsimd must stay responsive for the per-step causal masks.
    kpad_sb = []
    for h in range(HPC):
        t = p_qk.tile([128, S], BF16, tag=f"kpad{h}", name=f"kpad{h}")
        eng = nc.vector if h < 2 else nc.gpsimd
        eng.memset(t[64 * (1 - h % 2):64 * (1 - h % 2) + 64, :], 0.0)
        kpad_sb.append(t)

    # constants for the matmul-based causal mask of diagonal score tiles:
    # s_diag += (neg_ident.T @ upper01) = -30000 where k > q.
    ones_t = p_cst.tile([128, 128], BF16, tag="ones")
    nc.vector.memset(ones_t[:, :], 1.0)
    upper01 = p_cst.tile([128, 128], BF16, tag="upper01")
    nc.gpsimd.affine_select(upper01[:, :], ones_t[:, :], pattern=[[-1, 128]],
                            compare_op=ALU.is_ge, fill=0.0, base=-1,
                            channel_multiplier=1)   # keep where k - q - 1 >= 0
    lower_t = p_cst.tile([128, 128], BF16, tag="lower")
    nc.gpsimd.affine_select(lower_t[:, :], ones_t[:, :], pattern=[[1, 128]],
                            compare_op=ALU.is_ge, fill=0.0, base=0,
                            channel_multiplier=-1)  # keep where q - k >= 0
    ident_t = p_cst.tile([128, 128], BF16, tag="ident")
    nc.gpsimd.affine_select(ident_t[:, :], lower_t[:, :], pattern=[[-1, 128]],
                            compare_op=ALU.is_ge, fill=0.0, base=0,
                            channel_multiplier=1)   # and k - q >= 0
    neg_ident = p_cst.tile([128, 128], BF16, tag="neg_ident")
    nc.vector.tensor_scalar_mul(neg_ident[:, :], ident_t[:, :], MASK_NEG)

    qk_sb = []   # [q01, q23], bf16 [128, S] each (post-rope)
    for hp in range(2):
        qk_sb.append(p_qk.tile([128, S], BF16, tag=f"qT{hp}", name=f"qT{hp}"))
    vx_sb = [None] * NT  # [128, 4*65] bf16: per head 64 v-cols + ones col
    aT_sb = [p_aT.tile([128, S], BF16, tag=f"aT{i}", name=f"aT{i}")
             for i in range(2)]

    # ---- PSUM pools (8 banks):
    #   ps_s  2 x [128,1024] f32 = 4 banks   (scores double-buffer)
    #   ps_o  2 x [65,512]   f32 = 2 banks   (attention output accumulator,
    #         two 1-bank halves per block; each half's bank frees at its own
    #         fq close, so the next block's PVs rarely wait)
    #   ps_f  2 x [128,512]  f32 = 2 banks   (filler units: qk/v/y matmuls)
    ps_s = ctx.enter_context(tc.tile_pool(name="ps_s", bufs=2, space="PSUM"))
    ps_o = ctx.enter_context(tc.tile_pool(name="ps_o", bufs=2, space="PSUM"))
    ps_f = ctx.enter_context(tc.tile_pool(name="ps_f", bufs=2, space="PSUM"))

    # ---- filler units ----
    def qk_unit(mc, ns):
        # one [128,512] column block of the q/k projection + rope.
        # mc: 0=q01 1=q23 2=k01 3=k23; ns: 512-col block of seq.
        def emit():
            with nc.named_scope(f"qk{mc}_{ns}"):
                qk_ps = ps_f.tile([128, 512], F32, tag="f", name=f"qk{mc}_{ns}")
                g, off = GOFF[mc]
                for kc in range(8):
                    nc.tensor.matmul(
                        qk_ps[:, :],
                        w4[:, g, kc, ds(off, 128)],
                        x4[:, ns // 2, kc, ds((ns % 2) * 512, 512)],
                        start=(kc == 0), stop=(kc == 7))
                raw = p_tmp.tile([128, 512], BF16, tag="raw")
                nc.vector.tensor_scalar_add(raw[:, :], qk_ps[:, :],
                                            bqk_sb[:, mc:mc + 1])
                swp = p_tmp.tile([128, 512], BF16, tag="swp")
                nc.vector.stream_shuffle(swp[:, :], raw[:, :], SWAP16)
                t1 = p_tmp.tile([128, 512], BF16, tag="t1")
                nc.vector.tensor_mul(t1[:, :], swp[:, :], tabQ[:, ts(ns, 512)])
                t2 = p_tmp.tile([128, 512], BF16, tag="t2")
                nc.vector.tensor_mul(t2[:, :], raw[:, :], tabP[:, ts(ns, 512)])
                if mc < 2:
                    nc.vector.tensor_add(
                        qk_sb[mc][:, ts(ns, 512)], t1[:, :], t2[:, :])
                else:
                    hp = mc - 2
                    for hr in range(2):
                        nc.vector.tensor_add(
                            kpad_sb[2 * hp + hr][64 * hr:64 * hr + 64,
                                                 ts(ns, 512)],
                            t1[64 * hr:64 * hr + 64, :],
                            t2[64 * hr:64 * hr + 64, :])
        return emit

    def v_unit(st):
        # v projection for one 128-seq tile (all 4 heads) + ones column.
        def emit():
            with nc.named_scope(f"v{st}"):
                v_ps = ps_f.tile([128, 512], F32, tag="f", name=f"v{st}")
                for kc in range(8):
                    nc.tensor.matmul(
                        v_ps[:, 0:256],
                        x4[:, st // 8, kc, ds((st % 8) * 128, 128)],
                        w4[:, 1, kc, :],
                        start=(kc == 0), stop=(kc == 7))
                vx_t = p_vx.tile([128, HPC * 65], BF16, tag=f"vx{st}",
                                 name=f"vx{st}")
                vv = vx_t.rearrange("p (h c) -> p h c", c=65)
                nc.vector.memset(vv[:, :, 64:65], 1.0)
                nc.vector.tensor_add(
                    vv[:, :, 0:64],
                    v_ps[:, 0:256].rearrange("p (h c) -> p h c", c=64)[:, :, :],
                    bvb_sb.rearrange("p (h c) -> p h c", c=64)[:, :, :])
                vx_sb[st] = vx_t
        return emit

    def y_unit(qt):
        # output projection + store for one 128-seq tile.
        def emit():
            with nc.named_scope(f"y{qt}"):
                y_sb = p_y.tile([128, 1024], BF16, tag="ysb")
                for nh in range(2):
                    y_ps = ps_f.tile([128, 512], F32, tag="f", name=f"y{qt}_{nh}")
                    for kc in range(2):
                        nc.tensor.matmul(
                            y_ps[:, :],
                            aT_sb[kc][:, ts(qt, 128)],
                            wout_sb[:, ds(kc * 1024 + nh * 512, 512)],
                            start=(kc == 0), stop=(kc == 1))
                    if nh == 0:
                        nc.vector.tensor_copy(y_sb[:, ts(nh, 512)],
                                              y_ps[:, :])
                    else:
                        nc.scalar.copy(y_sb[:, ts(nh, 512)], y_ps[:, :])
                eng = nc.sync if qt < 8 else nc.scalar
                eng.dma_start(y[ts(qt, 128), :], y_sb[:, :])
        return emit

    fillers = []       # deque of emit closures

    def pull_fillers(n):
        for _ in range(min(n, len(fillers))):
            fillers.pop(0)()

    # ---- attention ----
    pend = [None]   # (p_t, j, q0, w, oT, qlo, h)

    def emit_pv(pv):
        p_t, j, q0, w, oT, qlo, h = pv
        c0 = (q0 - qlo) * 128
        pos = c0
        while pos < c0 + w:
            nxt = min((pos // 512 + 1) * 512, c0 + w)
            hi = pos // 512          # which oT half-tile
            gbank = (qlo * 128 + pos) // 512
            nc.tensor.matmul(
                oT[hi][:, ds(pos - 512 * hi, nxt - pos)],
                vx_sb[j][:, ds(65 * h, 65)],
                p_t[:, ds(pos - c0, nxt - pos)],
                start=(j == 0), stop=(j == 4 * gbank + 3),
                skip_group_check=True)
            pos = nxt

    def finish_quarter(h, hp, hr, half, oT_c, c):
        # Normalize one 512-col half-tile of oT as soon as its accumulation
        # group closed.  den goes out via a gpsimd-queue DMA transposed to
        # [128,4] so the reciprocal runs across partitions (~0.1us instead
        # of ~3.3us); gpsimd's ring is kept clear of bulk traffic so the
        # round-trip latency stays low.
        # den+num copies run on ACT: at fq time ACT is either ahead (half-0)
        # or stalled on the next block's scores (block end), while the DVE
        # queue is deep in filler rope chains -- this frees the oT bank
        # ~2us sooner and keeps the eviction off the strict PE queue path.
        tg = f"{h}_{half}_{c}"
        den = p_r.tile([1, 512], F32, tag="den", name=f"den{tg}")
        nc.scalar.copy(den[:, :], oT_c[64:65, :])
        denT = p_r.tile([128, 4], F32, tag="denT", name=f"denT{tg}")
        nc.sync.dma_start(denT[:, :], den[:, :])
        num = p_r.tile([64, 512], BF16, tag="num", name=f"num{tg}")
        nc.scalar.copy(num[:, :], oT_c[0:64, :])
        recT = p_r.tile([128, 4], F32, tag="recT", name=f"recT{tg}")
        nc.vector.reciprocal(recT[:, :], denT[:, :])
        rrow = p_r.tile([1, 512], F32, tag="rrow", name=f"rrow{tg}")
        nc.sync.dma_start(rrow[:, :], recT[:, :])
        rb = p_r.tile([64, 512], F32, tag="rb", name=f"rb{tg}")
        nc.gpsimd.partition_broadcast(rb[:, :], rrow[:, :])
        nc.vector.tensor_mul(
            aT_sb[hp][64 * hr:64 * hr + 64, ds(1024 * half + 512 * c, 512)],
            num[:, :], rb[:, :])

    # deferred second-half fq of the previous block: flushed right after
    # its carried last PV, inside the next block's first step
    pend_fq = [None]   # (h, hp, hr, half, oT_b)

    def flush_pend():
        # emit the carried last PV + deferred fq(1) of the previous block
        if pend[0] is not None:
            emit_pv(pend[0])
            pend[0] = None
        if pend_fq[0] is not None:
            finish_quarter(*pend_fq[0], 1)
            pend_fq[0] = None

    def attn_block(h, half, fill_every=2, late=()):
        hp, hr = h // 2, h % 2
        qT = qk_sb[hp]
        kT = kpad_sb[h]
        qlo, qhi = 8 * half, 8 * half + 8   # q-tile range
        jc0 = qlo + 3                       # PV(jc0) closes oT cols 0..511
        with nc.named_scope(f"attn_h{h}_{half}"):
            oT = (ps_o.tile([65, 512], F32, tag="oT", name=f"oT{h}_{half}a"),
                  ps_o.tile([65, 512], F32, tag="oT", name=f"oT{h}_{half}b"))
            for j in range(qhi):
                q0 = max(j, qlo)
                w = (qhi - q0) * 128
                s_ps = ps_s.tile([128, 1024], F32, tag="s")
                diag = (q0 == j)
                for n0 in range(0, w, 512):
                    nn = min(512, w - n0)
                    has_mask = diag and n0 == 0
                    nc.tensor.matmul(
                        s_ps[:, ds(n0, nn)],
                        kT[:, ts(j, 128)],
                        qT[:, ds(q0 * 128 + n0, nn)],
                        start=True, stop=not has_mask,
                        skip_group_check=True)
                    if has_mask:
                        nc.tensor.matmul(
                            s_ps[:, 0:128], neg_ident[:, :],
                            upper01[:, :], start=False, stop=True,
                            skip_group_check=True)
                p_t = p_p.tile([128, 1024], BF16, tag="p")
                nc.scalar.activation(
                    p_t[:, 0:w], s_ps[:, 0:w], AF.Exp, scale=SCALE)
                if j % fill_every == 0:
                    pull_fillers(1)
                if pend[0] is not None:
                    emit_pv(pend[0])
                    if pend[0][1] == jc0:
                        # first quarter's accumulation closed: normalize it
                        # now so its consumers (y tiles) can interleave early.
                        finish_quarter(h, hp, hr, half, oT[0], 0)
                        fillers.extend(late)
                pend[0] = (p_t, j, q0, w, oT, qlo, h)
            # end of block: flush the last PV now.  No filler pull here --
            # filler matmuls between the flush and the next block's scores
            # would sit in the boundary critical path of the strict-FIFO
            # PE queue.
            emit_pv(pend[0])
            pend[0] = None
            finish_quarter(h, hp, hr, half, oT[1], 1)

    # ---- emission schedule ----
    with nc.named_scope("warmup"):
        # ~8us of junk matmuls: bridges the DMA ramp so the PE clock is
        # warm (and stays warm) when the first projection waves arrive.
        wu = ps_f.tile([128, 128], F32, tag="f", name="warmup")
        for r in range(110):
            nc.tensor.matmul(wu[:, :], junk[:, :], junk[:, :],
                             start=(r == 0), stop=(r == 109),
                             skip_group_check=True)

    # front-load: q01 for seq 0..1023, k01 for seq 0..511 and v for seq
    # 0..127 -- the minimum for the first attention block's first steps;
    # everything else is fillers.
    for u in (qk_unit(0, 0), qk_unit(0, 1), qk_unit(2, 0), v_unit(0)):
        u()

    # fillers, in dependency-safe pull order.  The filler DVE chains (rope,
    # ~2.3us per qk unit) are the scarce resource during the projection
    # phase, so qk pulls are spread into the ACT-heavy half-1 blocks where
    # the vector engine has slack; h1_1 needs no new fillers itself, so it
    # runs 4th and absorbs the q23/k23 chains for h2/h3.
    fillers.extend([qk_unit(2, 1)])
    fillers.extend([v_unit(st) for st in range(1, 8)])
    fillers.extend([qk_unit(0, 2), qk_unit(0, 3),
                    v_unit(8), v_unit(9), v_unit(10),
                    qk_unit(2, 2), qk_unit(2, 3)])
    fillers.extend([v_unit(st) for st in range(11, 16)])
    fillers.extend([qk_unit(1, 0), qk_unit(1, 1),
                    qk_unit(3, 0), qk_unit(3, 1),
                    qk_unit(1, 2), qk_unit(1, 3),
                    qk_unit(3, 2), qk_unit(3, 3)])

    attn_block(0, 0, fill_every=1)
    attn_block(1, 0, fill_every=2)
    attn_block(0, 1, fill_every=2)
    attn_block(1, 1, fill_every=2)
    attn_block(2, 0, fill_every=2)
    attn_block(3, 0, fill_every=2)
    # all of y0..7 becomes eligible at h3_0's end; pulling them during
    # h2_1 (not via a late hook right behind the gating fq chain) keeps
    # their aT dependencies clear of the strict-FIFO PE queue.
    fillers.extend([y_unit(qt) for qt in range(0, 8)])
    attn_block(2, 1, fill_every=2)
    attn_block(3, 1, fill_every=2)
    # Tail: junk matmuls cover the fq-chain wait ahead of the final y
    # tiles in the PE queue so the clock stays at 2.4GHz and the y matmuls
    # run warm.
    with nc.named_scope("warmup2"):
        wu2 = ps_f.tile([128, 128], F32, tag="f", name="warmup2")
        for r in range(80):
            nc.tensor.matmul(wu2[:, :], junk[:, :], junk[:, :],
                             start=(r == 0), stop=(r == 79),
                             skip_group_check=True)
    pull_fillers(len(fillers))
    for qt in range(8, 16):
        y_unit(qt)()


def build():
    nc = bacc.Bacc("TRN2", target_bir_lowering=False, debug=False,
                   num_devices=NCORES)
    xh = nc.dram_tensor("xh", [128, 8 * S], BF16, kind="ExternalInput").ap()
    wg = nc.dram_tensor("wg", [128, 3 * 2048], BF16,
                        kind="ExternalInput").ap()
    wout = nc.dram_tensor("wout", [128, 2048], BF16, kind="ExternalInput").ap()
    bqk = nc.dram_tensor("bqk", [128, 4], F32, kind="ExternalInput").ap()
    bvb = nc.dram_tensor("bvb", [128, 256], F32, kind="ExternalInput").ap()
    ropeP = nc.dram_tensor("ropeP", [128, S], BF16, kind="ExternalInput").ap()
    ropeQ = nc.dram_tensor("ropeQ", [128, S], BF16, kind="ExternalInput").ap()
    y = nc.dram_tensor("y", [S, D], BF16, kind="ExternalOutput").ap()

    from contextlib import ExitStack
    with tile.TileContext(nc) as tc:
        with ExitStack() as ctx:
            _body(ctx, tc, (xh, wg, wout, bqk, bvb, ropeP, ropeQ), (y,))
    nc.compile()
    return nc


# per-head column permutation: [e0..15, o0..15, e16..31, o16..31] so the rope
# partner of every row lives in the same 32-partition quadrant.
_PERM64 = np.concatenate([
    np.arange(0, 32, 2), np.arange(1, 32, 2),
    np.arange(32, 64, 2), np.arange(33, 64, 2)])


def make_core_inputs(x, rope_cos, rope_sin, Wqkv, bqkv, Wout, bout, core):
    """Build the per-core device input map (numpy, host-side sharding)."""
    b, g = core // HPC, core % HPC
    heads = [HPC * g + i for i in range(HPC)]
    bf = ml_dtypes.bfloat16

    # x pre-arranged to the SBUF layout [p, half, kc, 1024]: one flat
    # [128, 16384] tensor so every device DMA is a contiguous 2D row copy.
    xT = np.ascontiguousarray(x[b].T)                       # [D, S]
    xh_np = (xT.reshape(8, 128, 2, 1024).transpose(1, 2, 0, 3)
             .reshape(128, 8 * S))

    # w columns: [q01 | k01 | v | q23 | k23]; q/k within each head _PERM64
    # order, v unpermuted.  Then pre-arranged to [p, grp, kc, 256] with
    # grp 0 = q01|k01, 1 = v, 2 = q23|k23 -> flat [128, 6144].
    qcols, kcols = [], []
    for h in heads:
        qcols.append(Wqkv[:, 0 * D + 64 * h + _PERM64])
        kcols.append(Wqkv[:, 1 * D + 64 * h + _PERM64])
    vcols = [Wqkv[:, 2 * D + 64 * h:2 * D + 64 * h + 64] for h in heads]
    w_all_np = np.concatenate(
        [qcols[0], qcols[1], kcols[0], kcols[1]] + vcols +
        [qcols[2], qcols[3], kcols[2], kcols[3]], axis=1)   # [D, 768]
    wk = w_all_np.reshape(8, 128, 768).transpose(1, 0, 2)   # [p, kc, 768]
    wg_np = np.concatenate(
        [wk[:, :, 0:256].reshape(128, 2048),
         wk[:, :, 256:512].reshape(128, 2048),
         wk[:, :, 512:768].reshape(128, 2048)], axis=1)     # [128, 6144]

    bq = [bqkv[0 * D + 64 * h + _PERM64] for h in heads]
    bk = [bqkv[1 * D + 64 * h + _PERM64] for h in heads]
    bqk_np = np.stack([np.concatenate([bq[0], bq[1]]),
                       np.concatenate([bq[2], bq[3]]),
                       np.concatenate([bk[0], bk[1]]),
                       np.concatenate([bk[2], bk[3]])], axis=1)

    bv = np.concatenate(
        [bqkv[2 * D + 64 * h:2 * D + 64 * h + 64] for h in heads])
    bvb_np = np.tile(bv[None, :], (128, 1)).astype(np.float32)

    wout_rows = np.concatenate(
        [Wout[64 * h:64 * h + 64, :] for h in heads], axis=0)  # [256, D]
    wout_np = np.concatenate([wout_rows[0:128, :], wout_rows[128:256, :]],
                             axis=1)  # [128, 2048]

    cosT = np.ascontiguousarray(rope_cos.T).astype(np.float32)  # [32, S]
    sinT = np.ascontiguousarray(rope_sin.T).astype(np.float32)
    p64 = np.concatenate([cosT[0:16], cosT[0:16], cosT[16:32], cosT[16:32]])
    q64 = np.concatenate([-sinT[0:16], sinT[0:16], -sinT[16:32], sinT[16:32]])
    ropeP_np = np.tile(p64, (2, 1))
    ropeQ_np = np.tile(q64, (2, 1))

    return {
        "xh": np.ascontiguousarray(xh_np).astype(bf),
        "wg": np.ascontiguousarray(wg_np).astype(bf),
        "wout": np.ascontiguousarray(wout_np).astype(bf),
        "bqk": np.ascontiguousarray(bqk_np).astype(np.float32),
        "bvb": bvb_np,
        "ropeP": np.ascontiguousarray(ropeP_np).astype(bf),
        "ropeQ": np.ascontiguousarray(ropeQ_np).astype(bf),
    }


_NC_CACHE = None


def kernel(x, rope_cos, rope_sin, Wqkv, bqkv, Wout, bout):
    global _NC_CACHE, LAST_RESULTS
    x = np.asarray(x, dtype=np.float32)
    rope_cos = np.asarray(rope_cos, dtype=np.float32)
    rope_sin = np.asarray(rope_sin, dtype=np.float32)
    Wqkv = np.asarray(Wqkv, dtype=np.float32)
    bqkv = np.asarray(bqkv, dtype=np.float32)
    Wout = np.asarray(Wout, dtype=np.float32)
    bout = np.asarray(bout, dtype=np.float32)

    if _NC_CACHE is None:
        _NC_CACHE = build()
    nc = _NC_CACHE

    in_maps = [
        make_core_inputs(x, rope_cos, rope_sin, Wqkv, bqkv, Wout, bout, c)
        for c in range(NCORES)
    ]
    res = run_bass_kernel_spmd(nc, in_maps, core_ids=list(range(NCORES)),
                               trace=TRACE)
    LAST_RESULTS = res

    out = np.zeros((B, S, D), dtype=np.float32)
    for c in range(NCORES):
        out[c // HPC] += res.results[c]["y"].astype(np.float32)
    out += bout[None, None, :]
    return out



# revision 36
# speedup vs baseline: 1.1520x; 1.1520x over previous
# Causal self-attention kernel for 8 Trainium2 NeuronCores (Bass/Tile).
#
# Sharding: core c -> batch b = c//4, head group g = c%4 (heads 4g..4g+3).
# Each core computes the qkv projection for its batch restricted to its heads
# (column-sharded Wqkv), rope, causal flash attention for its 4 heads, and a
# row-sharded output projection producing a partial [S, D] bf16 output.  The
# host sums the 4 partials per batch and adds bout.
#
# The kernel is organized as a single software-pipelined stream: the scalar
# engine's softmax exp is the attention-phase bottleneck (~80us of ACT time vs
# ~58us of attention matmul), so the qkv projection waves, v projection and
# output projection are emitted as PE "filler" units interleaved between
# attention steps instead of as separate serial phases.  Attention begins on
# the first head as soon as its q/k columns are roped (~11us in), while the
# rest of x is still loading.
#
# Device-side notes:
#  * All matmul inputs are bf16; accumulation f32 in PSUM.
#  * x is pre-transposed on host to xT [D, S]; loaded criticality-first as
#    merged 3D DMAs (seq-half 0 on sync, seq-half 1 on gpsimd); w loads in
#    column-group order [qk01 | v | qk23] on scalar so the first projection
#    waves start as early as possible.  Attention blocks alternate small
#    (half-0) and big (half-1) so projection fillers land where ACT is the
#    local bottleneck.
#  * q/k are produced directly transposed (qT/kT [dims, S]).  Within each head
#    the dims are permuted to [e0..15, o0..15, e16..31, o16..31] so each rope
#    partner lives in the same 32-partition quadrant: the even/odd swap is a
#    single DVE stream_shuffle (mask = swap 16-halves) instead of SBUF DMAs.
#  * Scores are computed transposed, sT[k, q], with the k-side stationary
#    operand zero-padded to K=128 (K=64 matmuls never warm the PE clock gate).
#  * Causal masking of diagonal score tiles is one extra accumulating matmul
#    (-30000 * upper-triangle).
#  * Softmax without max-subtraction: p = exp(s/8) straight out of PSUM on the
#    scalar engine, bf16 out.  The scalar engine does (almost) nothing else.
#  * v_ext [k, 65] carries a ones-column so the PV matmul accumulates the
#    softmax denominator as row 64 of oT [65, q].
#  * Softmax denominators are reciprocated via a DMA transpose [1,1024] ->
#    [128,8] so the (slow, serial-in-free-dim) DVE reciprocal runs across
#    partitions: ~0.1us instead of ~6.6us per head-half.
#  * y partials are written bf16 (host accumulates in f32); stores spread
#    through the attention stream instead of draining at the end.

import numpy as np
import ml_dtypes

import concourse.bass as bass
import concourse.tile as tile
import concourse.mybir as mybir
from concourse import bacc
from concourse.bass import ts, ds
from concourse.bass_utils import run_bass_kernel_spmd

BF16 = mybir.dt.bfloat16
F32 = mybir.dt.float32
AF = mybir.ActivationFunctionType
ALU = mybir.AluOpType

B, S, D = 2, 2048, 1024
H, HD = 16, 64
NCORES = 8
HPC = 4            # heads per core
NT = S // 128      # 16 seq tiles
SCALE = HD ** -0.5
MASK_NEG = -30000.0

SWAP16 = [16 + i for i in range(16)] + list(range(16))  # stream_shuffle mask

# Module-level knobs / results (used by test.py).
TRACE = False
LAST_RESULTS = None


def _body(ctx, tc, ins, outs):
    nc = tc.nc
    xh, wg, wout, bqk, bvb, ropeP, ropeQ = ins
    (y,) = outs

    # ---- SBUF pools ----
    p_x = ctx.enter_context(tc.tile_pool(name="x", bufs=1))
    p_w = ctx.enter_context(tc.tile_pool(name="w", bufs=1))
    p_cst = ctx.enter_context(tc.tile_pool(name="cst", bufs=1))
    p_qk = ctx.enter_context(tc.tile_pool(name="qk", bufs=1))
    p_vx = ctx.enter_context(tc.tile_pool(name="vx", bufs=1))
    p_aT = ctx.enter_context(tc.tile_pool(name="aT", bufs=1))
    p_tmp = ctx.enter_context(tc.tile_pool(name="tmp", bufs=3))
    p_p = ctx.enter_context(tc.tile_pool(name="p", bufs=6))
    p_r = ctx.enter_context(tc.tile_pool(name="r", bufs=2))
    p_y = ctx.enter_context(tc.tile_pool(name="y", bufs=4))

    # ---- PE warmup fodder: memset junk immediately (no DMA dependency) ----
    junk = p_cst.tile([128, 128], BF16, tag="junk")
    nc.vector.memset(junk[:, :], 0.25)

    # ---- input DMA issues ----
    # The host pre-arranges x and w into the exact SBUF layout, so every
    # load is a pure 2D contiguous-row DMA (128 descriptors, ~0.8us issue;
    # multi-row 3D APs cost ~5-9us of HWDGE issue time on the engine).
    # Rings stream descriptors in issue order and fair-share HBM, so the
    # ~4MB critical set (x half-0, w qk01 cols, rope, bqk) leads every ring.
    #   xh layout: [p, half, kc, 1024]  ->  x_t2 cols (half*8 + kc)*1024
    #   wg layout: [p, grp, kc, 256], grp 0 = q01|k01, 1 = v, 2 = q23|k23
    x_t2 = p_x.tile([128, 8 * S], BF16, tag="x", name="x")
    x4 = x_t2.rearrange("p (h kc s) -> p h kc s", kc=8, s=1024)
    w_t2 = p_w.tile([128, 3 * 2048], BF16, tag="w", name="w")
    w4 = w_t2.rearrange("p (g kc c) -> p g kc c", kc=8, c=256)
    tabP = p_cst.tile([128, S], BF16, tag="tabP")
    tabQ = p_cst.tile([128, S], BF16, tag="tabQ")
    bqk_sb = p_cst.tile([128, 4], F32, tag="bqk")
    bvb_sb = p_cst.tile([128, 256], F32, tag="bvb")
    wout_sb = p_w.tile([128, 2048], BF16, tag="wout")
    # Ring order IS the priority: within a ring, transfers complete in
    # issue order, and concurrent rings fair-share HBM.  Bulk that isn't
    # needed until late (x half-1, wout, w23) goes BEHIND the critical set
    # on the same rings instead of competing from a third ring.  gpsimd's
    # SWDGE ring stays clear for the latency-sensitive fq round-trips and
    # the y stores.
    nc.sync.dma_start(x_t2[:, 0:4096], xh[:, 0:4096])
    nc.scalar.dma_start(bqk_sb[:, :], bqk[:, :])
    nc.scalar.dma_start(w_t2[:, 0:2048], wg[:, 0:2048])
    nc.sync.dma_start(x_t2[:, 4096:8192], xh[:, 4096:8192])
    nc.scalar.dma_start(tabQ[:, 0:1024], ropeQ[:, 0:1024])
    nc.scalar.dma_start(tabP[:, 0:1024], ropeP[:, 0:1024])
    nc.scalar.dma_start(w_t2[:, 2048:4096], wg[:, 2048:4096])
    nc.scalar.dma_start(bvb_sb[:, :], bvb[:, :])
    nc.scalar.dma_start(tabQ[:, 1024:2048], ropeQ[:, 1024:2048])
    nc.scalar.dma_start(tabP[:, 1024:2048], ropeP[:, 1024:2048])
    # bulk remainder, strictly behind the critical set on the same rings
    nc.sync.dma_start(wout_sb[:, :], wout[:, :])
    nc.scalar.dma_start(x_t2[:, 8192:16384], xh[:, 8192:16384])
    nc.scalar.dma_start(w_t2[:, 4096:6144], wg[:, 4096:6144])
    # (group, col offset) in w4 for each qk projection unit column block
    GOFF = {0: (0, 0), 2: (0, 128), 1: (2, 0), 3: (2, 128)}

    # per-head zero-padded kT [128, S]: only head h's 64 rows (at offset
    # 64*(h%2)) are nonzero, for full-K scores matmuls.  The memsets run on
    # the DVE, which is otherwise idle until the first projection evacuates
    # (~14us); gpsimd must stay responsive for the per-step causal masks.
    kpad_sb = []
    for h in range(HPC):
        t = p_qk.tile([128, S], BF16, tag=f"kpad{h}", name=f"kpad{h}")
        eng = nc.vector if h < 2 else nc.gpsimd
        eng.memset(t[64 * (1 - h % 2):64 * (1 - h % 2) + 64, :], 0.0)
        kpad_sb.append(t)

    # constants for the matmul-based causal mask of diagonal score tiles:
    # s_diag += (neg_ident.T @ upper01) = -30000 where k > q.
    ones_t = p_cst.tile([128, 128], BF16, tag="ones")
    nc.vector.memset(ones_t[:, :], 1.0)
    upper01 = p_cst.tile([128, 128], BF16, tag="upper01")
    nc.gpsimd.affine_select(upper01[:, :], ones_t[:, :], pattern=[[-1, 128]],
                            compare_op=ALU.is_ge, fill=0.0, base=-1,
                            channel_multiplier=1)   # keep where k - q - 1 >= 0
    lower_t = p_cst.tile([128, 128], BF16, tag="lower")
    nc.gpsimd.affine_select(lower_t[:, :], ones_t[:, :], pattern=[[1, 128]],
                            compare_op=ALU.is_ge, fill=0.0, base=0,
                            channel_multiplier=-1)  # keep where q - k >= 0
    ident_t = p_cst.tile([128, 128], BF16, tag="ident")
    nc.gpsimd.affine_select(ident_t[:, :], lower_t[:, :], pattern=[[-1, 128]],
                            compare_op=ALU.is_ge, fill=0.0, base=0,
                            channel_multiplier=1)   # and k - q >= 0
    neg_ident = p_cst.tile([128, 128], BF16, tag="neg_ident")
    nc.vector.tensor_scalar_mul(neg_ident[:, :], ident_t[:, :], MASK_NEG)

    qk_sb = []   # [q01, q23], bf16 [128, S] each (post-rope)
    for hp in range(2):
        qk_sb.append(p_qk.tile([128, S], BF16, tag=f"qT{hp}", name=f"qT{hp}"))
    vx_sb = [None] * NT  # [128, 4*65] bf16: per head 64 v-cols + ones col
    aT_sb = [p_aT.tile([128, S], BF16, tag=f"aT{i}", name=f"aT{i}")
             for i in range(2)]

    # ---- PSUM pools (8 banks):
    #   ps_s  2 x [128,1024] f32 = 4 banks   (scores double-buffer)
    #   ps_o  2 x [65,512]   f32 = 2 banks   (attention output accumulator,
    #         two 1-bank halves per block; each half's bank frees at its own
    #         fq close, so the next block's PVs rarely wait)
    #   ps_f  2 x [128,512]  f32 = 2 banks   (filler units: qk/v/y matmuls)
    ps_s = ctx.enter_context(tc.tile_pool(name="ps_s", bufs=2, space="PSUM"))
    ps_o = ctx.enter_context(tc.tile_pool(name="ps_o", bufs=2, space="PSUM"))
    ps_f = ctx.enter_context(tc.tile_pool(name="ps_f", bufs=2, space="PSUM"))

    # ---- filler units ----
    def qk_unit(mc, ns):
        # one [128,512] column block of the q/k projection + rope.
        # mc: 0=q01 1=q23 2=k01 3=k23; ns: 512-col block of seq.
        def emit():
            with nc.named_scope(f"qk{mc}_{ns}"):
                qk_ps = ps_f.tile([128, 512], F32, tag="f", name=f"qk{mc}_{ns}")
                g, off = GOFF[mc]
                for kc in range(8):
                    nc.tensor.matmul(
                        qk_ps[:, :],
                        w4[:, g, kc, ds(off, 128)],
                        x4[:, ns // 2, kc, ds((ns % 2) * 512, 512)],
                        start=(kc == 0), stop=(kc == 7))
                raw = p_tmp.tile([128, 512], BF16, tag="raw")
                nc.vector.tensor_scalar_add(raw[:, :], qk_ps[:, :],
                                            bqk_sb[:, mc:mc + 1])
                swp = p_tmp.tile([128, 512], BF16, tag="swp")
                nc.vector.stream_shuffle(swp[:, :], raw[:, :], SWAP16)
                t1 = p_tmp.tile([128, 512], BF16, tag="t1")
                nc.vector.tensor_mul(t1[:, :], swp[:, :], tabQ[:, ts(ns, 512)])
                t2 = p_tmp.tile([128, 512], BF16, tag="t2")
                nc.vector.tensor_mul(t2[:, :], raw[:, :], tabP[:, ts(ns, 512)])
                if mc < 2:
                    nc.vector.tensor_add(
                        qk_sb[mc][:, ts(ns, 512)], t1[:, :], t2[:, :])
                else:
                    hp = mc - 2
                    for hr in range(2):
                        nc.vector.tensor_add(
                            kpad_sb[2 * hp + hr][64 * hr:64 * hr + 64,
                                                 ts(ns, 512)],
                            t1[64 * hr:64 * hr + 64, :],
                            t2[64 * hr:64 * hr + 64, :])
        return emit

    def v_unit(st):
        # v projection for one 128-seq tile (all 4 heads) + ones column.
        def emit():
            with nc.named_scope(f"v{st}"):
                v_ps = ps_f.tile([128, 512], F32, tag="f", name=f"v{st}")
                for kc in range(8):
                    nc.tensor.matmul(
                        v_ps[:, 0:256],
                        x4[:, st // 8, kc, ds((st % 8) * 128, 128)],
                        w4[:, 1, kc, :],
                        start=(kc == 0), stop=(kc == 7))
                vx_t = p_vx.tile([128, HPC * 65], BF16, tag=f"vx{st}",
                                 name=f"vx{st}")
                vv = vx_t.rearrange("p (h c) -> p h c", c=65)
                nc.vector.memset(vv[:, :, 64:65], 1.0)
                nc.vector.tensor_add(
                    vv[:, :, 0:64],
                    v_ps[:, 0:256].rearrange("p (h c) -> p h c", c=64)[:, :, :],
                    bvb_sb.rearrange("p (h c) -> p h c", c=64)[:, :, :])
                vx_sb[st] = vx_t
        return emit

    def y_unit(qt):
        # output projection + store for one 128-seq tile.
        def emit():
            with nc.named_scope(f"y{qt}"):
                y_sb = p_y.tile([128, 1024], BF16, tag="ysb")
                for nh in range(2):
                    y_ps = ps_f.tile([128, 512], F32, tag="f", name=f"y{qt}_{nh}")
                    for kc in range(2):
                        nc.tensor.matmul(
                            y_ps[:, :],
                            aT_sb[kc][:, ts(qt, 128)],
                            wout_sb[:, ds(kc * 1024 + nh * 512, 512)],
                            start=(kc == 0), stop=(kc == 1))
                    if nh == 0:
                        nc.vector.tensor_copy(y_sb[:, ts(nh, 512)],
                                              y_ps[:, :])
                    else:
                        nc.scalar.copy(y_sb[:, ts(nh, 512)], y_ps[:, :])
                eng = nc.sync if qt < 8 else nc.scalar
                eng.dma_start(y[ts(qt, 128), :], y_sb[:, :])
        return emit

    fillers = []       # deque of emit closures

    def pull_fillers(n):
        for _ in range(min(n, len(fillers))):
            fillers.pop(0)()

    # ---- attention ----
    pend = [None]   # (p_t, j, q0, w, oT, qlo, h)

    def emit_pv(pv):
        p_t, j, q0, w, oT, qlo, h = pv
        c0 = (q0 - qlo) * 128
        pos = c0
        while pos < c0 + w:
            nxt = min((pos // 512 + 1) * 512, c0 + w)
            hi = pos // 512          # which oT half-tile
            gbank = (qlo * 128 + pos) // 512
            nc.tensor.matmul(
                oT[hi][:, ds(pos - 512 * hi, nxt - pos)],
                vx_sb[j][:, ds(65 * h, 65)],
                p_t[:, ds(pos - c0, nxt - pos)],
                start=(j == 0), stop=(j == 4 * gbank + 3),
                skip_group_check=True)
            pos = nxt

    def finish_quarter(h, hp, hr, half, oT_c, c):
        # Normalize one 512-col half-tile of oT as soon as its accumulation
        # group closed.  den goes out via a gpsimd-queue DMA transposed to
        # [128,4] so the reciprocal runs across partitions (~0.1us instead
        # of ~3.3us); gpsimd's ring is kept clear of bulk traffic so the
        # round-trip latency stays low.
        # den+num copies run on ACT: at fq time ACT is either ahead (half-0)
        # or stalled on the next block's scores (block end), while the DVE
        # queue is deep in filler rope chains -- this frees the oT bank
        # ~2us sooner and keeps the eviction off the strict PE queue path.
        tg = f"{h}_{half}_{c}"
        den = p_r.tile([1, 512], F32, tag="den", name=f"den{tg}")
        nc.scalar.copy(den[:, :], oT_c[64:65, :])
        denT = p_r.tile([128, 4], F32, tag="denT", name=f"denT{tg}")
        nc.sync.dma_start(denT[:, :], den[:, :])
        num = p_r.tile([64, 512], BF16, tag="num", name=f"num{tg}")
        nc.scalar.copy(num[:, :], oT_c[0:64, :])
        recT = p_r.tile([128, 4], F32, tag="recT", name=f"recT{tg}")
        nc.vector.reciprocal(recT[:, :], denT[:, :])
        rrow = p_r.tile([1, 512], F32, tag="rrow", name=f"rrow{tg}")
        nc.sync.dma_start(rrow[:, :], recT[:, :])
        rb = p_r.tile([64, 512], F32, tag="rb", name=f"rb{tg}")
        nc.gpsimd.partition_broadcast(rb[:, :], rrow[:, :])
        nc.vector.tensor_mul(
            aT_sb[hp][64 * hr:64 * hr + 64, ds(1024 * half + 512 * c, 512)],
            num[:, :], rb[:, :])

    # deferred second-half fq of the previous block: flushed right after
    # its carried last PV, inside the next block's first step
    pend_fq = [None]   # (h, hp, hr, half, oT_b)

    def flush_pend():
        # emit the carried last PV + deferred fq(1) of the previous block
        if pend[0] is not None:
            emit_pv(pend[0])
            pend[0] = None
        if pend_fq[0] is not None:
            finish_quarter(*pend_fq[0], 1)
            pend_fq[0] = None

    def attn_block(h, half, fill_every=2, late=()):
        hp, hr = h // 2, h % 2
        qT = qk_sb[hp]
        kT = kpad_sb[h]
        qlo, qhi = 8 * half, 8 * half + 8   # q-tile range
        jc0 = qlo + 3                       # PV(jc0) closes oT cols 0..511
        with nc.named_scope(f"attn_h{h}_{half}"):
            oT = (ps_o.tile([65, 512], F32, tag="oT", name=f"oT{h}_{half}a"),
                  ps_o.tile([65, 512], F32, tag="oT", name=f"oT{h}_{half}b"))
            for j in range(qhi):
                q0 = max(j, qlo)
                w = (qhi - q0) * 128
                s_ps = ps_s.tile([128, 1024], F32, tag="s")
                diag = (q0 == j)
                for n0 in range(0, w, 512):
                    nn = min(512, w - n0)
                    has_mask = diag and n0 == 0
                    nc.tensor.matmul(
                        s_ps[:, ds(n0, nn)],
                        kT[:, ts(j, 128)],
                        qT[:, ds(q0 * 128 + n0, nn)],
                        start=True, stop=not has_mask,
                        skip_group_check=True)
                    if has_mask:
                        nc.tensor.matmul(
                            s_ps[:, 0:128], neg_ident[:, :],
                            upper01[:, :], start=False, stop=True,
                            skip_group_check=True)
                p_t = p_p.tile([128, 1024], BF16, tag="p")
                nc.scalar.activation(
                    p_t[:, 0:w], s_ps[:, 0:w], AF.Exp, scale=SCALE)
                if j % fill_every == 0:
                    pull_fillers(1)
                if pend[0] is not None:
                    emit_pv(pend[0])
                    if pend[0][1] == jc0:
                        # first quarter's accumulation closed: normalize it
                        # now so its consumers (y tiles) can interleave early.
                        finish_quarter(h, hp, hr, half, oT[0], 0)
                        fillers.extend(late)
                pend[0] = (p_t, j, q0, w, oT, qlo, h)
            # end of block: flush the last PV now, then let fillers cover
            # the evacuation latency.
            emit_pv(pend[0])
            pend[0] = None
            finish_quarter(h, hp, hr, half, oT[1], 1)
            pull_fillers(1)

    # ---- emission schedule ----
    with nc.named_scope("warmup"):
        # ~8us of junk matmuls: bridges the DMA ramp so the PE clock is
        # warm (and stays warm) when the first projection waves arrive.
        wu = ps_f.tile([128, 128], F32, tag="f", name="warmup")
        for r in range(110):
            nc.tensor.matmul(wu[:, :], junk[:, :], junk[:, :],
                             start=(r == 0), stop=(r == 109),
                             skip_group_check=True)

    # front-load: q01 for seq 0..1023, k01 for seq 0..511 and v for seq
    # 0..127 -- the minimum for the first attention block's first steps;
    # everything else is fillers.
    for u in (qk_unit(0, 0), qk_unit(0, 1), qk_unit(2, 0), v_unit(0)):
        u()

    # fillers, in dependency-safe pull order.  The filler DVE chains (rope,
    # ~2.3us per qk unit) are the scarce resource during the projection
    # phase, so qk pulls are spread into the ACT-heavy half-1 blocks where
    # the vector engine has slack; h1_1 needs no new fillers itself, so it
    # runs 4th and absorbs the q23/k23 chains for h2/h3.
    fillers.extend([qk_unit(2, 1)])
    fillers.extend([v_unit(st) for st in range(1, 8)])
    fillers.extend([qk_unit(0, 2), qk_unit(0, 3),
                    v_unit(8), v_unit(9), v_unit(10),
                    qk_unit(2, 2), qk_unit(2, 3)])
    fillers.extend([v_unit(st) for st in range(11, 16)])
    fillers.extend([qk_unit(1, 0), qk_unit(1, 1),
                    qk_unit(3, 0), qk_unit(3, 1),
                    qk_unit(1, 2), qk_unit(1, 3),
                    qk_unit(3, 2), qk_unit(3, 3)])

    attn_block(0, 0, fill_every=1)
    attn_block(1, 0, fill_every=2)
    attn_block(0, 1, fill_every=2)
    attn_block(1, 1, fill_every=2)
    attn_block(2, 0, fill_every=2)
    attn_block(3, 0, fill_every=2)
    # all of y0..7 becomes eligible at h3_0's end; pulling them during
    # h2_1 (not via a late hook right behind the gating fq chain) keeps
    # their aT dependencies clear of the strict-FIFO PE queue.
    fillers.extend([y_unit(qt) for qt in range(0, 8)])
    attn_block(2, 1, fill_every=2)
    attn_block(3, 1, fill_every=2)
    # Tail: junk matmuls cover the fq-chain wait ahead of the final y
    # tiles in the PE queue so the clock stays at 2.4GHz and the y matmuls
    # run warm.
    with nc.named_scope("warmup2"):
        wu2 = ps_f.tile([128, 128], F32, tag="f", name="warmup2")
        for r in range(80):
            nc.tensor.matmul(wu2[:, :], junk[:, :], junk[:, :],
                             start=(r == 0), stop=(r == 79),
                             skip_group_check=True)
    pull_fillers(len(fillers))
    for qt in range(8, 16):
        y_unit(qt)()


def build():
    nc = bacc.Bacc("TRN2", target_bir_lowering=False, debug=False,
                   num_devices=NCORES)
    xh = nc.dram_tensor("xh", [128, 8 * S], BF16, kind="ExternalInput").ap()
    wg = nc.dram_tensor("wg", [128, 3 * 2048], BF16,
                        kind="ExternalInput").ap()
    wout = nc.dram_tensor("wout", [128, 2048], BF16, kind="ExternalInput").ap()
    bqk = nc.dram_tensor("bqk", [128, 4], F32, kind="ExternalInput").ap()
    bvb = nc.dram_tensor("bvb", [128, 256], F32, kind="ExternalInput").ap()
    ropeP = nc.dram_tensor("ropeP", [128, S], BF16, kind="ExternalInput").ap()
    ropeQ = nc.dram_tensor("ropeQ", [128, S], BF16, kind="ExternalInput").ap()
    y = nc.dram_tensor("y", [S, D], BF16, kind="ExternalOutput").ap()

    from contextlib import ExitStack
    with tile.TileContext(nc) as tc:
        with ExitStack() as ctx:
            _body(ctx, tc, (xh, wg, wout, bqk, bvb, ropeP, ropeQ), (y,))
    nc.compile()
    return nc


# per-head column permutation: [e0..15, o0..15, e16..31, o16..31] so the rope
# partner of every row lives in the same 32-partition quadrant.
_PERM64 = np.concatenate([
    np.arange(0, 32, 2), np.arange(1, 32, 2),
    np.arange(32, 64, 2), np.arange(33, 64, 2)])


def make_core_inputs(x, rope_cos, rope_sin, Wqkv, bqkv, Wout, bout, core):
    """Build the per-core device input map (numpy, host-side sharding)."""
    b, g = core // HPC, core % HPC
    heads = [HPC * g + i for i in range(HPC)]
    bf = ml_dtypes.bfloat16

    # x pre-arranged to the SBUF layout [p, half, kc, 1024]: one flat
    # [128, 16384] tensor so every device DMA is a contiguous 2D row copy.
    xT = np.ascontiguousarray(x[b].T)                       # [D, S]
    xh_np = (xT.reshape(8, 128, 2, 1024).transpose(1, 2, 0, 3)
             .reshape(128, 8 * S))

    # w columns: [q01 | k01 | v | q23 | k23]; q/k within each head _PERM64
    # order, v unpermuted.  Then pre-arranged to [p, grp, kc, 256] with
    # grp 0 = q01|k01, 1 = v, 2 = q23|k23 -> flat [128, 6144].
    qcols, kcols = [], []
    for h in heads:
        qcols.append(Wqkv[:, 0 * D + 64 * h + _PERM64])
        kcols.append(Wqkv[:, 1 * D + 64 * h + _PERM64])
    vcols = [Wqkv[:, 2 * D + 64 * h:2 * D + 64 * h + 64] for h in heads]
    w_all_np = np.concatenate(
        [qcols[0], qcols[1], kcols[0], kcols[1]] + vcols +
        [qcols[2], qcols[3], kcols[2], kcols[3]], axis=1)   # [D, 768]
    wk = w_all_np.reshape(8, 128, 768).transpose(1, 0, 2)   # [p, kc, 768]
    wg_np = np.concatenate(
        [wk[:, :, 0:256].reshape(128, 2048),
         wk[:, :, 256:512].reshape(128, 2048),
         wk[:, :, 512:768].reshape(128, 2048)], axis=1)     # [128, 6144]

    bq = [bqkv[0 * D + 64 * h + _PERM64] for h in heads]
    bk = [bqkv[1 * D + 64 * h + _PERM64] for h in heads]
    bqk_np = np.stack([np.concatenate([bq[0], bq[1]]),
                       np.concatenate([bq[2], bq[3]]),
                       np.concatenate([bk[0], bk[1]]),
                       np.concatenate([bk[2], bk[3]])], axis=1)

    bv = np.concatenate(
        [bqkv[2 * D + 64 * h:2 * D + 64 * h + 64] for h in heads])
    bvb_np = np.tile(bv[None, :], (128, 1)).astype(np.float32)

    wout_rows = np.concatenate(
        [Wout[64 * h:64 * h + 64, :] for h in heads], axis=0)  # [256, D]
    wout_np = np.concatenate([wout_rows[0:128, :], wout_rows[128:256, :]],
                             axis=1)  # [128, 2048]

    cosT = np.ascontiguousarray(rope_cos.T).astype(np.float32)  # [32, S]
    sinT = np.ascontiguousarray(rope_sin.T).astype(np.float32)
    p64 = np.concatenate([cosT[0:16], cosT[0:16], cosT[16:32], cosT[16:32]])
    q64 = np.concatenate([-sinT[0:16], sinT[0:16], -sinT[16:32], sinT[16:32]])
    ropeP_np = np.tile(p64, (2, 1))
    ropeQ_np = np.tile(q64, (2, 1))

    return {
        "xh": np.ascontiguousarray(xh_np).astype(bf),
        "wg": np.ascontiguousarray(wg_np).astype(bf),
        "wout": np.ascontiguousarray(wout_np).astype(bf),
        "bqk": np.ascontiguousarray(bqk_np).astype(np.float32),
        "bvb": bvb_np,
        "ropeP": np.ascontiguousarray(ropeP_np).astype(bf),
        "ropeQ": np.ascontiguousarray(ropeQ_np).astype(bf),
    }


_NC_CACHE = None


def kernel(x, rope_cos, rope_sin, Wqkv, bqkv, Wout, bout):
    global _NC_CACHE, LAST_RESULTS
    x = np.asarray(x, dtype=np.float32)
    rope_cos = np.asarray(rope_cos, dtype=np.float32)
    rope_sin = np.asarray(rope_sin, dtype=np.float32)
    Wqkv = np.asarray(Wqkv, dtype=np.float32)
    bqkv = np.asarray(bqkv, dtype=np.float32)
    Wout = np.asarray(Wout, dtype=np.float32)
    bout = np.asarray(bout, dtype=np.float32)

    if _NC_CACHE is None:
        _NC_CACHE = build()
    nc = _NC_CACHE

    in_maps = [
        make_core_inputs(x, rope_cos, rope_sin, Wqkv, bqkv, Wout, bout, c)
        for c in range(NCORES)
    ]
    res = run_bass_kernel_spmd(nc, in_maps, core_ids=list(range(NCORES)),
                               trace=TRACE)
    LAST_RESULTS = res

    out = np.zeros((B, S, D), dtype=np.float32)
    for c in range(NCORES):
        out[c // HPC] += res.results[c]["y"].astype(np.float32)
    out += bout[None, None, :]
    return out



# revision 37
# speedup vs baseline: 1.1538x; 1.0016x over previous
# Causal self-attention kernel for 8 Trainium2 NeuronCores (Bass/Tile).
#
# Sharding: core c -> batch b = c//4, head group g = c%4 (heads 4g..4g+3).
# Each core computes the qkv projection for its batch restricted to its heads
# (column-sharded Wqkv), rope, causal flash attention for its 4 heads, and a
# row-sharded output projection producing a partial [S, D] bf16 output.  The
# host sums the 4 partials per batch and adds bout.
#
# The kernel is organized as a single software-pipelined stream: the scalar
# engine's softmax exp is the attention-phase bottleneck (~80us of ACT time vs
# ~58us of attention matmul), so the qkv projection waves, v projection and
# output projection are emitted as PE "filler" units interleaved between
# attention steps instead of as separate serial phases.  Attention begins on
# the first head as soon as its q/k columns are roped (~11us in), while the
# rest of x is still loading.
#
# Device-side notes:
#  * All matmul inputs are bf16; accumulation f32 in PSUM.
#  * x is pre-transposed on host to xT [D, S]; loaded criticality-first as
#    merged 3D DMAs (seq-half 0 on sync, seq-half 1 on gpsimd); w loads in
#    column-group order [qk01 | v | qk23] on scalar so the first projection
#    waves start as early as possible.  Attention blocks alternate small
#    (half-0) and big (half-1) so projection fillers land where ACT is the
#    local bottleneck.
#  * q/k are produced directly transposed (qT/kT [dims, S]).  Within each head
#    the dims are permuted to [e0..15, o0..15, e16..31, o16..31] so each rope
#    partner lives in the same 32-partition quadrant: the even/odd swap is a
#    single DVE stream_shuffle (mask = swap 16-halves) instead of SBUF DMAs.
#  * Scores are computed transposed, sT[k, q], with the k-side stationary
#    operand zero-padded to K=128 (K=64 matmuls never warm the PE clock gate).
#  * Causal masking of diagonal score tiles is one extra accumulating matmul
#    (-30000 * upper-triangle).
#  * Softmax without max-subtraction: p = exp(s/8) straight out of PSUM on the
#    scalar engine, bf16 out.  The scalar engine does (almost) nothing else.
#  * v_ext [k, 65] carries a ones-column so the PV matmul accumulates the
#    softmax denominator as row 64 of oT [65, q].
#  * Softmax denominators are reciprocated via a DMA transpose [1,1024] ->
#    [128,8] so the (slow, serial-in-free-dim) DVE reciprocal runs across
#    partitions: ~0.1us instead of ~6.6us per head-half.
#  * y partials are written bf16 (host accumulates in f32); stores spread
#    through the attention stream instead of draining at the end.

import numpy as np
import ml_dtypes

import concourse.bass as bass
import concourse.tile as tile
import concourse.mybir as mybir
from concourse import bacc
from concourse.bass import ts, ds
from concourse.bass_utils import run_bass_kernel_spmd

BF16 = mybir.dt.bfloat16
F32 = mybir.dt.float32
AF = mybir.ActivationFunctionType
ALU = mybir.AluOpType

B, S, D = 2, 2048, 1024
H, HD = 16, 64
NCORES = 8
HPC = 4            # heads per core
NT = S // 128      # 16 seq tiles
SCALE = HD ** -0.5
MASK_NEG = -30000.0

SWAP16 = [16 + i for i in range(16)] + list(range(16))  # stream_shuffle mask

# Module-level knobs / results (used by test.py).
TRACE = False
LAST_RESULTS = None


def _body(ctx, tc, ins, outs):
    nc = tc.nc
    xh, wg, wout, bqk, bvb, ropeP, ropeQ = ins
    (y,) = outs

    # ---- SBUF pools ----
    p_x = ctx.enter_context(tc.tile_pool(name="x", bufs=1))
    p_w = ctx.enter_context(tc.tile_pool(name="w", bufs=1))
    p_cst = ctx.enter_context(tc.tile_pool(name="cst", bufs=1))
    p_qk = ctx.enter_context(tc.tile_pool(name="qk", bufs=1))
    p_vx = ctx.enter_context(tc.tile_pool(name="vx", bufs=1))
    p_aT = ctx.enter_context(tc.tile_pool(name="aT", bufs=1))
    p_tmp = ctx.enter_context(tc.tile_pool(name="tmp", bufs=3))
    p_p = ctx.enter_context(tc.tile_pool(name="p", bufs=6))
    p_r = ctx.enter_context(tc.tile_pool(name="r", bufs=2))
    p_y = ctx.enter_context(tc.tile_pool(name="y", bufs=4))

    # ---- PE warmup fodder: memset junk immediately (no DMA dependency) ----
    junk = p_cst.tile([128, 128], BF16, tag="junk")
    nc.vector.memset(junk[:, :], 0.25)

    # ---- input DMA issues ----
    # The host pre-arranges x and w into the exact SBUF layout, so every
    # load is a pure 2D contiguous-row DMA (128 descriptors, ~0.8us issue;
    # multi-row 3D APs cost ~5-9us of HWDGE issue time on the engine).
    # Rings stream descriptors in issue order and fair-share HBM, so the
    # ~4MB critical set (x half-0, w qk01 cols, rope, bqk) leads every ring.
    #   xh layout: [p, half, kc, 1024]  ->  x_t2 cols (half*8 + kc)*1024
    #   wg layout: [p, grp, kc, 256], grp 0 = q01|k01, 1 = v, 2 = q23|k23
    x_t2 = p_x.tile([128, 8 * S], BF16, tag="x", name="x")
    x4 = x_t2.rearrange("p (h kc s) -> p h kc s", kc=8, s=1024)
    w_t2 = p_w.tile([128, 3 * 2048], BF16, tag="w", name="w")
    w4 = w_t2.rearrange("p (g kc c) -> p g kc c", kc=8, c=256)
    tabP = p_cst.tile([128, S], BF16, tag="tabP")
    tabQ = p_cst.tile([128, S], BF16, tag="tabQ")
    bqk_sb = p_cst.tile([128, 4], F32, tag="bqk")
    bvb_sb = p_cst.tile([128, 256], F32, tag="bvb")
    wout_sb = p_w.tile([128, 2048], BF16, tag="wout")
    # Ring order IS the priority: within a ring, transfers complete in
    # issue order, and concurrent rings fair-share HBM.  Bulk that isn't
    # needed until late (x half-1, wout, w23) goes BEHIND the critical set
    # on the same rings instead of competing from a third ring.  gpsimd's
    # SWDGE ring stays clear for the latency-sensitive fq round-trips and
    # the y stores.
    nc.sync.dma_start(x_t2[:, 0:4096], xh[:, 0:4096])
    nc.scalar.dma_start(bqk_sb[:, :], bqk[:, :])
    nc.scalar.dma_start(w_t2[:, 0:2048], wg[:, 0:2048])
    nc.sync.dma_start(x_t2[:, 4096:8192], xh[:, 4096:8192])
    nc.scalar.dma_start(tabQ[:, 0:1024], ropeQ[:, 0:1024])
    nc.scalar.dma_start(tabP[:, 0:1024], ropeP[:, 0:1024])
    nc.scalar.dma_start(w_t2[:, 2048:4096], wg[:, 2048:4096])
    nc.scalar.dma_start(bvb_sb[:, :], bvb[:, :])
    nc.scalar.dma_start(tabQ[:, 1024:2048], ropeQ[:, 1024:2048])
    nc.scalar.dma_start(tabP[:, 1024:2048], ropeP[:, 1024:2048])
    # bulk remainder, strictly behind the critical set on the same rings
    nc.sync.dma_start(wout_sb[:, :], wout[:, :])
    nc.scalar.dma_start(x_t2[:, 8192:16384], xh[:, 8192:16384])
    nc.scalar.dma_start(w_t2[:, 4096:6144], wg[:, 4096:6144])
    # (group, col offset) in w4 for each qk projection unit column block
    GOFF = {0: (0, 0), 2: (0, 128), 1: (2, 0), 3: (2, 128)}

    # per-head zero-padded kT [128, S]: only head h's 64 rows (at offset
    # 64*(h%2)) are nonzero, for full-K scores matmuls.  The memsets run on
    # the DVE, which is otherwise idle until the first projection evacuates
    # (~14us); gpsimd must stay responsive for the per-step causal masks.
    kpad_sb = []
    for h in range(HPC):
        t = p_qk.tile([128, S], BF16, tag=f"kpad{h}", name=f"kpad{h}")
        eng = nc.vector if h < 2 else nc.gpsimd
        eng.memset(t[64 * (1 - h % 2):64 * (1 - h % 2) + 64, :], 0.0)
        kpad_sb.append(t)

    # constants for the matmul-based causal mask of diagonal score tiles:
    # s_diag += (neg_ident.T @ upper01) = -30000 where k > q.
    ones_t = p_cst.tile([128, 128], BF16, tag="ones")
    nc.vector.memset(ones_t[:, :], 1.0)
    upper01 = p_cst.tile([128, 128], BF16, tag="upper01")
    nc.gpsimd.affine_select(upper01[:, :], ones_t[:, :], pattern=[[-1, 128]],
                            compare_op=ALU.is_ge, fill=0.0, base=-1,
                            channel_multiplier=1)   # keep where k - q - 1 >= 0
    lower_t = p_cst.tile([128, 128], BF16, tag="lower")
    nc.gpsimd.affine_select(lower_t[:, :], ones_t[:, :], pattern=[[1, 128]],
                            compare_op=ALU.is_ge, fill=0.0, base=0,
                            channel_multiplier=-1)  # keep where q - k >= 0
    ident_t = p_cst.tile([128, 128], BF16, tag="ident")
    nc.gpsimd.affine_select(ident_t[:, :], lower_t[:, :], pattern=[[-1, 128]],
                            compare_op=ALU.is_ge, fill=0.0, base=0,
                            channel_multiplier=1)   # and k - q >= 0
    neg_ident = p_cst.tile([128, 128], BF16, tag="neg_ident")
    nc.vector.tensor_scalar_mul(neg_ident[:, :], ident_t[:, :], MASK_NEG)

    qk_sb = []   # [q01, q23], bf16 [128, S] each (post-rope)
    for hp in range(2):
        qk_sb.append(p_qk.tile([128, S], BF16, tag=f"qT{hp}", name=f"qT{hp}"))
    vx_sb = [None] * NT  # [128, 4*65] bf16: per head 64 v-cols + ones col
    aT_sb = [p_aT.tile([128, S], BF16, tag=f"aT{i}", name=f"aT{i}")
             for i in range(2)]

    # ---- PSUM pools (8 banks):
    #   ps_s  2 x [128,1024] f32 = 4 banks   (scores double-buffer)
    #   ps_o  2 x [65,512]   f32 = 2 banks   (attention output accumulator,
    #         two 1-bank halves per block; each half's bank frees at its own
    #         fq close, so the next block's PVs rarely wait)
    #   ps_f  2 x [128,512]  f32 = 2 banks   (filler units: qk/v/y matmuls)
    ps_s = ctx.enter_context(tc.tile_pool(name="ps_s", bufs=2, space="PSUM"))
    ps_o = ctx.enter_context(tc.tile_pool(name="ps_o", bufs=2, space="PSUM"))
    ps_f = ctx.enter_context(tc.tile_pool(name="ps_f", bufs=2, space="PSUM"))

    # ---- filler units ----
    def qk_unit(mc, ns):
        # one [128,512] column block of the q/k projection + rope.
        # mc: 0=q01 1=q23 2=k01 3=k23; ns: 512-col block of seq.
        def emit():
            with nc.named_scope(f"qk{mc}_{ns}"):
                qk_ps = ps_f.tile([128, 512], F32, tag="f", name=f"qk{mc}_{ns}")
                g, off = GOFF[mc]
                for kc in range(8):
                    nc.tensor.matmul(
                        qk_ps[:, :],
                        w4[:, g, kc, ds(off, 128)],
                        x4[:, ns // 2, kc, ds((ns % 2) * 512, 512)],
                        start=(kc == 0), stop=(kc == 7))
                raw = p_tmp.tile([128, 512], BF16, tag="raw")
                nc.vector.tensor_scalar_add(raw[:, :], qk_ps[:, :],
                                            bqk_sb[:, mc:mc + 1])
                swp = p_tmp.tile([128, 512], BF16, tag="swp")
                nc.vector.stream_shuffle(swp[:, :], raw[:, :], SWAP16)
                t1 = p_tmp.tile([128, 512], BF16, tag="t1")
                nc.vector.tensor_mul(t1[:, :], swp[:, :], tabQ[:, ts(ns, 512)])
                t2 = p_tmp.tile([128, 512], BF16, tag="t2")
                nc.vector.tensor_mul(t2[:, :], raw[:, :], tabP[:, ts(ns, 512)])
                if mc < 2:
                    nc.vector.tensor_add(
                        qk_sb[mc][:, ts(ns, 512)], t1[:, :], t2[:, :])
                else:
                    hp = mc - 2
                    for hr in range(2):
                        nc.vector.tensor_add(
                            kpad_sb[2 * hp + hr][64 * hr:64 * hr + 64,
                                                 ts(ns, 512)],
                            t1[64 * hr:64 * hr + 64, :],
                            t2[64 * hr:64 * hr + 64, :])
        return emit

    def v_unit(st):
        # v projection for one 128-seq tile (all 4 heads) + ones column.
        def emit():
            with nc.named_scope(f"v{st}"):
                v_ps = ps_f.tile([128, 512], F32, tag="f", name=f"v{st}")
                for kc in range(8):
                    nc.tensor.matmul(
                        v_ps[:, 0:256],
                        x4[:, st // 8, kc, ds((st % 8) * 128, 128)],
                        w4[:, 1, kc, :],
                        start=(kc == 0), stop=(kc == 7))
                vx_t = p_vx.tile([128, HPC * 65], BF16, tag=f"vx{st}",
                                 name=f"vx{st}")
                vv = vx_t.rearrange("p (h c) -> p h c", c=65)
                nc.vector.memset(vv[:, :, 64:65], 1.0)
                nc.vector.tensor_add(
                    vv[:, :, 0:64],
                    v_ps[:, 0:256].rearrange("p (h c) -> p h c", c=64)[:, :, :],
                    bvb_sb.rearrange("p (h c) -> p h c", c=64)[:, :, :])
                vx_sb[st] = vx_t
        return emit

    def y_unit(qt):
        # output projection + store for one 128-seq tile.
        def emit():
            with nc.named_scope(f"y{qt}"):
                y_sb = p_y.tile([128, 1024], BF16, tag="ysb")
                for nh in range(2):
                    y_ps = ps_f.tile([128, 512], F32, tag="f", name=f"y{qt}_{nh}")
                    for kc in range(2):
                        nc.tensor.matmul(
                            y_ps[:, :],
                            aT_sb[kc][:, ts(qt, 128)],
                            wout_sb[:, ds(kc * 1024 + nh * 512, 512)],
                            start=(kc == 0), stop=(kc == 1))
                    if nh == 0:
                        nc.vector.tensor_copy(y_sb[:, ts(nh, 512)],
                                              y_ps[:, :])
                    else:
                        nc.scalar.copy(y_sb[:, ts(nh, 512)], y_ps[:, :])
                eng = nc.sync if qt < 8 else nc.scalar
                eng.dma_start(y[ts(qt, 128), :], y_sb[:, :])
        return emit

    fillers = []       # deque of emit closures

    def pull_fillers(n):
        for _ in range(min(n, len(fillers))):
            fillers.pop(0)()

    # ---- attention ----
    pend = [None]   # (p_t, j, q0, w, oT, qlo, h)

    def emit_pv(pv):
        p_t, j, q0, w, oT, qlo, h = pv
        c0 = (q0 - qlo) * 128
        pos = c0
        while pos < c0 + w:
            nxt = min((pos // 512 + 1) * 512, c0 + w)
            hi = pos // 512          # which oT half-tile
            gbank = (qlo * 128 + pos) // 512
            nc.tensor.matmul(
                oT[hi][:, ds(pos - 512 * hi, nxt - pos)],
                vx_sb[j][:, ds(65 * h, 65)],
                p_t[:, ds(pos - c0, nxt - pos)],
                start=(j == 0), stop=(j == 4 * gbank + 3),
                skip_group_check=True)
            pos = nxt

    def finish_quarter(h, hp, hr, half, oT_c, c):
        # Normalize one 512-col half-tile of oT as soon as its accumulation
        # group closed.  den goes out via a gpsimd-queue DMA transposed to
        # [128,4] so the reciprocal runs across partitions (~0.1us instead
        # of ~3.3us); gpsimd's ring is kept clear of bulk traffic so the
        # round-trip latency stays low.
        # den+num copies run on ACT: at fq time ACT is either ahead (half-0)
        # or stalled on the next block's scores (block end), while the DVE
        # queue is deep in filler rope chains -- this frees the oT bank
        # ~2us sooner and keeps the eviction off the strict PE queue path.
        tg = f"{h}_{half}_{c}"
        den = p_r.tile([1, 512], F32, tag="den", name=f"den{tg}")
        nc.scalar.copy(den[:, :], oT_c[64:65, :])
        denT = p_r.tile([128, 4], F32, tag="denT", name=f"denT{tg}")
        nc.sync.dma_start(denT[:, :], den[:, :])
        num = p_r.tile([64, 512], BF16, tag="num", name=f"num{tg}")
        nc.scalar.copy(num[:, :], oT_c[0:64, :])
        recT = p_r.tile([128, 4], F32, tag="recT", name=f"recT{tg}")
        nc.vector.reciprocal(recT[:, :], denT[:, :])
        rrow = p_r.tile([1, 512], F32, tag="rrow", name=f"rrow{tg}")
        nc.sync.dma_start(rrow[:, :], recT[:, :])
        rb = p_r.tile([64, 512], F32, tag="rb", name=f"rb{tg}")
        nc.gpsimd.partition_broadcast(rb[:, :], rrow[:, :])
        nc.vector.tensor_mul(
            aT_sb[hp][64 * hr:64 * hr + 64, ds(1024 * half + 512 * c, 512)],
            num[:, :], rb[:, :])

    # deferred second-half fq of the previous block: flushed right after
    # its carried last PV, inside the next block's first step
    pend_fq = [None]   # (h, hp, hr, half, oT_b)

    def flush_pend():
        # emit the carried last PV + deferred fq(1) of the previous block
        if pend[0] is not None:
            emit_pv(pend[0])
            pend[0] = None
        if pend_fq[0] is not None:
            finish_quarter(*pend_fq[0], 1)
            pend_fq[0] = None

    def attn_block(h, half, fill_every=2, late=()):
        hp, hr = h // 2, h % 2
        qT = qk_sb[hp]
        kT = kpad_sb[h]
        qlo, qhi = 8 * half, 8 * half + 8   # q-tile range
        jc0 = qlo + 3                       # PV(jc0) closes oT cols 0..511
        with nc.named_scope(f"attn_h{h}_{half}"):
            oT = (ps_o.tile([65, 512], F32, tag="oT", name=f"oT{h}_{half}a"),
                  ps_o.tile([65, 512], F32, tag="oT", name=f"oT{h}_{half}b"))
            for j in range(qhi):
                q0 = max(j, qlo)
                w = (qhi - q0) * 128
                s_ps = ps_s.tile([128, 1024], F32, tag="s")
                diag = (q0 == j)
                for n0 in range(0, w, 512):
                    nn = min(512, w - n0)
                    has_mask = diag and n0 == 0
                    nc.tensor.matmul(
                        s_ps[:, ds(n0, nn)],
                        kT[:, ts(j, 128)],
                        qT[:, ds(q0 * 128 + n0, nn)],
                        start=True, stop=not has_mask,
                        skip_group_check=True)
                    if has_mask:
                        nc.tensor.matmul(
                            s_ps[:, 0:128], neg_ident[:, :],
                            upper01[:, :], start=False, stop=True,
                            skip_group_check=True)
                p_t = p_p.tile([128, 1024], BF16, tag="p")
                nc.scalar.activation(
                    p_t[:, 0:w], s_ps[:, 0:w], AF.Exp, scale=SCALE)
                if j % fill_every == 0:
                    pull_fillers(1)
                if pend[0] is not None:
                    emit_pv(pend[0])
                    if pend[0][1] == jc0:
                        # first quarter's accumulation closed: normalize it
                        # now so its consumers (y tiles) can interleave early.
                        finish_quarter(h, hp, hr, half, oT[0], 0)
                        fillers.extend(late)
                pend[0] = (p_t, j, q0, w, oT, qlo, h)
            # end of block: flush the last PV now, then let fillers cover
            # the evacuation latency.
            emit_pv(pend[0])
            pend[0] = None
            finish_quarter(h, hp, hr, half, oT[1], 1)
            pull_fillers(1)

    # ---- emission schedule ----
    with nc.named_scope("warmup"):
        # ~8us of junk matmuls: bridges the DMA ramp so the PE clock is
        # warm (and stays warm) when the first projection waves arrive.
        wu = ps_f.tile([128, 128], F32, tag="f", name="warmup")
        for r in range(150):
            nc.tensor.matmul(wu[:, :], junk[:, :], junk[:, :],
                             start=(r == 0), stop=(r == 149),
                             skip_group_check=True)

    # front-load: q01 for seq 0..1023, k01 for seq 0..511 and v for seq
    # 0..127 -- the minimum for the first attention block's first steps;
    # everything else is fillers.
    for u in (qk_unit(0, 0), qk_unit(0, 1), qk_unit(2, 0), v_unit(0)):
        u()

    # fillers, in dependency-safe pull order.  The filler DVE chains (rope,
    # ~2.3us per qk unit) are the scarce resource during the projection
    # phase, so qk pulls are spread into the ACT-heavy half-1 blocks where
    # the vector engine has slack; h1_1 needs no new fillers itself, so it
    # runs 4th and absorbs the q23/k23 chains for h2/h3.
    fillers.extend([qk_unit(2, 1)])
    fillers.extend([v_unit(st) for st in range(1, 8)])
    fillers.extend([qk_unit(0, 2), qk_unit(0, 3),
                    v_unit(8), v_unit(9), v_unit(10),
                    qk_unit(2, 2), qk_unit(2, 3)])
    fillers.extend([v_unit(st) for st in range(11, 16)])
    fillers.extend([qk_unit(1, 0), qk_unit(1, 1),
                    qk_unit(3, 0), qk_unit(3, 1),
                    qk_unit(1, 2), qk_unit(1, 3),
                    qk_unit(3, 2), qk_unit(3, 3)])

    attn_block(0, 0, fill_every=1)
    attn_block(1, 0, fill_every=2)
    attn_block(0, 1, fill_every=2)
    attn_block(1, 1, fill_every=2)
    attn_block(2, 0, fill_every=2)
    attn_block(3, 0, fill_every=2)
    # all of y0..7 becomes eligible at h3_0's end; pulling them during
    # h2_1 (not via a late hook right behind the gating fq chain) keeps
    # their aT dependencies clear of the strict-FIFO PE queue.
    fillers.extend([y_unit(qt) for qt in range(0, 8)])
    attn_block(2, 1, fill_every=2)
    attn_block(3, 1, fill_every=2)
    # Tail: junk matmuls cover the fq-chain wait ahead of the final y
    # tiles in the PE queue so the clock stays at 2.4GHz and the y matmuls
    # run warm.
    with nc.named_scope("warmup2"):
        wu2 = ps_f.tile([128, 128], F32, tag="f", name="warmup2")
        for r in range(110):
            nc.tensor.matmul(wu2[:, :], junk[:, :], junk[:, :],
                             start=(r == 0), stop=(r == 109),
                             skip_group_check=True)
    pull_fillers(len(fillers))
    for qt in range(8, 16):
        y_unit(qt)()


def build():
    nc = bacc.Bacc("TRN2", target_bir_lowering=False, debug=False,
                   num_devices=NCORES)
    xh = nc.dram_tensor("xh", [128, 8 * S], BF16, kind="ExternalInput").ap()
    wg = nc.dram_tensor("wg", [128, 3 * 2048], BF16,
                        kind="ExternalInput").ap()
    wout = nc.dram_tensor("wout", [128, 2048], BF16, kind="ExternalInput").ap()
    bqk = nc.dram_tensor("bqk", [128, 4], F32, kind="ExternalInput").ap()
    bvb = nc.dram_tensor("bvb", [128, 256], F32, kind="ExternalInput").ap()
    ropeP = nc.dram_tensor("ropeP", [128, S], BF16, kind="ExternalInput").ap()
    ropeQ = nc.dram_tensor("ropeQ", [128, S], BF16, kind="ExternalInput").ap()
    y = nc.dram_tensor("y", [S, D], BF16, kind="ExternalOutput").ap()

    from contextlib import ExitStack
    with tile.TileContext(nc) as tc:
        with ExitStack() as ctx:
            _body(ctx, tc, (xh, wg, wout, bqk, bvb, ropeP, ropeQ), (y,))
    nc.compile()
    return nc


# per-head column permutation: [e0..15, o0..15, e16..31, o16..31] so the rope
# partner of every row lives in the same 32-partition quadrant.
_PERM64 = np.concatenate([
    np.arange(0, 32, 2), np.arange(1, 32, 2),
    np.arange(32, 64, 2), np.arange(33, 64, 2)])


def make_core_inputs(x, rope_cos, rope_sin, Wqkv, bqkv, Wout, bout, core):
    """Build the per-core device input map (numpy, host-side sharding)."""
    b, g = core // HPC, core % HPC
    heads = [HPC * g + i for i in range(HPC)]
    bf = ml_dtypes.bfloat16

    # x pre-arranged to the SBUF layout [p, half, kc, 1024]: one flat
    # [128, 16384] tensor so every device DMA is a contiguous 2D row copy.
    xT = np.ascontiguousarray(x[b].T)                       # [D, S]
    xh_np = (xT.reshape(8, 128, 2, 1024).transpose(1, 2, 0, 3)
             .reshape(128, 8 * S))

    # w columns: [q01 | k01 | v | q23 | k23]; q/k within each head _PERM64
    # order, v unpermuted.  Then pre-arranged to [p, grp, kc, 256] with
    # grp 0 = q01|k01, 1 = v, 2 = q23|k23 -> flat [128, 6144].
    qcols, kcols = [], []
    for h in heads:
        qcols.append(Wqkv[:, 0 * D + 64 * h + _PERM64])
        kcols.append(Wqkv[:, 1 * D + 64 * h + _PERM64])
    vcols = [Wqkv[:, 2 * D + 64 * h:2 * D + 64 * h + 64] for h in heads]
    w_all_np = np.concatenate(
        [qcols[0], qcols[1], kcols[0], kcols[1]] + vcols +
        [qcols[2], qcols[3], kcols[2], kcols[3]], axis=1)   # [D, 768]
    wk = w_all_np.reshape(8, 128, 768).transpose(1, 0, 2)   # [p, kc, 768]
    wg_np = np.concatenate(
        [wk[:, :, 0:256].reshape(128, 2048),
         wk[:, :, 256:512].reshape(128, 2048),
         wk[:, :, 512:768].reshape(128, 2048)], axis=1)     # [128, 6144]

    bq = [bqkv[0 * D + 64 * h + _PERM64] for h in heads]
    bk = [bqkv[1 * D + 64 * h + _PERM64] for h in heads]
    bqk_np = np.stack([np.concatenate([bq[0], bq[1]]),
                       np.concatenate([bq[2], bq[3]]),
                       np.concatenate([bk[0], bk[1]]),
                       np.concatenate([bk[2], bk[3]])], axis=1)

    bv = np.concatenate(
        [bqkv[2 * D + 64 * h:2 * D + 64 * h + 64] for h in heads])
    bvb_np = np.tile(bv[None, :], (128, 1)).astype(np.float32)

    wout_rows = np.concatenate(
        [Wout[64 * h:64 * h + 64, :] for h in heads], axis=0)  # [256, D]
    wout_np = np.concatenate([wout_rows[0:128, :], wout_rows[128:256, :]],
                             axis=1)  # [128, 2048]

    cosT = np.ascontiguousarray(rope_cos.T).astype(np.float32)  # [32, S]
    sinT = np.ascontiguousarray(rope_sin.T).astype(np.float32)
    p64 = np.concatenate([cosT[0:16], cosT[0:16], cosT[16:32], cosT[16:32]])
    q64 = np.concatenate([-sinT[0:16], sinT[0:16], -sinT[16:32], sinT[16:32]])
    ropeP_np = np.tile(p64, (2, 1))
    ropeQ_np = np.tile(q64, (2, 1))

    return {
        "xh": np.ascontiguousarray(xh_np).astype(bf),
        "wg": np.ascontiguousarray(wg_np).astype(bf),
        "wout": np.ascontiguousarray(wout_np).astype(bf),
        "bqk": np.ascontiguousarray(bqk_np).astype(np.float32),
        "bvb": bvb_np,
        "ropeP": np.ascontiguousarray(ropeP_np).astype(bf),
        "ropeQ": np.ascontiguousarray(ropeQ_np).astype(bf),
    }


_NC_CACHE = None


def kernel(x, rope_cos, rope_sin, Wqkv, bqkv, Wout, bout):
    global _NC_CACHE, LAST_RESULTS
    x = np.asarray(x, dtype=np.float32)
    rope_cos = np.asarray(rope_cos, dtype=np.float32)
    rope_sin = np.asarray(rope_sin, dtype=np.float32)
    Wqkv = np.asarray(Wqkv, dtype=np.float32)
    bqkv = np.asarray(bqkv, dtype=np.float32)
    Wout = np.asarray(Wout, dtype=np.float32)
    bout = np.asarray(bout, dtype=np.float32)

    if _NC_CACHE is None:
        _NC_CACHE = build()
    nc = _NC_CACHE

    in_maps = [
        make_core_inputs(x, rope_cos, rope_sin, Wqkv, bqkv, Wout, bout, c)
        for c in range(NCORES)
    ]
    res = run_bass_kernel_spmd(nc, in_maps, core_ids=list(range(NCORES)),
                               trace=TRACE)
    LAST_RESULTS = res

    out = np.zeros((B, S, D), dtype=np.float32)
    for c in range(NCORES):
        out[c // HPC] += res.results[c]["y"].astype(np.float32)
    out += bout[None, None, :]
    return out

